# revision 4
# baseline (speedup 1.0000x reference)
"""Trainium2 Bass kernel for nn_Cls_Loss_42331197670001.

Reference computation (N=128 samples, C=345 classes, A=512 features):
    dataW[n,c,:] = W[c] - W[labels[n]]
    sigma2[n,c]  = Lambda * dataW[n,c] @ Sigma[labels[n]] @ dataW[n,c]^T
    dW_dMean[n,c]= dataW[n,c] . (mean_target-mean_source)[labels[n]]
    aug = y_s + 0.5*sigma2 + Lambda*dW_dMean ;  loss = mean softmax-CE(aug, labels)

Everything depends on the sample n only through its label l, so the heavy
quadratic form is computed once per *unique* label:
    (W_c - W_l) Sigma_l (W_c - W_l)^T = 0.5*d_M(l,c) - b(l,c) + 0.5*s(l)
    d_M(l,c) = W_c M_l W_c^T          <- the only O(C*A*A) term, on device
    b, s, mean-shift, softmax-CE      <- tiny, host numpy in float64
where M_l = triu(S_l) + triu(S_l, 1) is the upper-triangular folding of the
symmetrized S_l = Sigma_l + Sigma_l^T (same quadratic form, 3/4 of the
matmul work at 256-row granularity since M[256:512, 0:256] == 0).

Device kernel (SPMD over 8 cores, unique labels sharded across cores),
per label j and per c-tile t (c = 345 classes in 3 tiles of 128):
    P[c,b] = sum_a Wt[a,c] M[a,b]    2 fp8 DoubleRow matmuls into PSUM:
                                      rows 0:256 x cols 0:512 (N=512) +
                                      rows 256:512 x cols 256:512 (N=256)
    d[c]   = sum_b P[c,b] W[c,b]     free-dim fused multiply-reduce, spread
                                      over the non-PE engines:
      t=0,1: DVE tensor_tensor_reduce straight out of PSUM (fp32)
      t=2:   ACT copy PSUM->SBUF bf16, then GPSIMD scalar_tensor_tensor
The stationary matmul operand is W^T (shared across labels) and d comes out
via accum_out, so the PE never runs reduction matmuls and the DVE/ACT/GPSIMD
legs run concurrently with the next label's matmuls.
"""

import math
import sys

import numpy as np

try:
    import concourse.bass as bass
except ImportError:  # harness runs from a bare directory
    sys.path.insert(0, "/opt/trn_rl_repo")
    import concourse.bass as bass

import ml_dtypes

import concourse.mybir as mybir
import concourse.tile as tile
from concourse import bacc
from concourse.bass import ts
from concourse.bass_utils import run_bass_kernel_spmd

N_CORES = 8
A = 512          # feature dim
C = 345          # class count
C_PAD = 384      # 3 * 128
C_TILES = 3

W_SCALE = 16.0
S_SCALE = 32.0
OUT_SCALE = W_SCALE * S_SCALE

FP8 = mybir.dt.float8e4
BF16 = mybir.dt.bfloat16
F32 = mybir.dt.float32
FP8_NP = ml_dtypes.float8_e4m3

MULT = mybir.AluOpType.mult
ADD = mybir.AluOpType.add
DR = mybir.MatmulPerfMode.DoubleRow


def build_nc(u_pc: int) -> bass.Bass:
    """Per core: u_pc labels; dout[p, t, j] = (W M_j W^T)[c,c] * OUT_SCALE
    for class c = 128*t + p."""
    nc = bacc.Bacc()
    wt8 = nc.dram_tensor("wt8", [128, 4, C_PAD], FP8, kind="ExternalInput")
    w32 = nc.dram_tensor("w32", [128, 2, A], F32, kind="ExternalInput")
    wb16 = nc.dram_tensor("wb16", [128, A], BF16, kind="ExternalInput")
    s01 = nc.dram_tensor("s01", [u_pc, 128, 2, A], FP8, kind="ExternalInput")
    s23 = nc.dram_tensor("s23", [u_pc, 128, 2, 256], FP8, kind="ExternalInput")
    dout = nc.dram_tensor("dout", [128, C_TILES, u_pc], F32, kind="ExternalOutput")

    with tile.TileContext(nc) as tc:
        with (
            tc.tile_pool(name="singles", bufs=1) as singles,
            tc.tile_pool(name="s01pool", bufs=4) as s01pool,
            tc.tile_pool(name="s23pool", bufs=4) as s23pool,
            tc.tile_pool(name="scr", bufs=4) as scrpool,
            tc.tile_pool(name="pcopy", bufs=3) as pcpool,
            tc.tile_pool(name="psum", bufs=6, space="PSUM") as ppool,
            tc.tile_pool(name="junkp", bufs=1, space="PSUM") as junkpool,
        ):
            wt8_sb = singles.tile([128, 4, C_PAD], FP8)
            nc.sync.dma_start(out=wt8_sb[:], in_=wt8[:])
            s01_first = s01pool.tile([128, 2, A], FP8, tag="s01")
            nc.sync.dma_start(out=s01_first[:], in_=s01[0])
            s23_first = s23pool.tile([128, 2, 256], FP8, tag="s23")
            nc.sync.dma_start(out=s23_first[:], in_=s23[0])
            w32_sb = singles.tile([128, 2, A], F32)
            nc.sync.dma_start(out=w32_sb[:, 0, :], in_=w32[:, 0, :])
            nc.sync.dma_start(out=w32_sb[:, 1, :], in_=w32[:, 1, :])
            wb16_sb = singles.tile([128, A], BF16)
            nc.sync.dma_start(out=wb16_sb[:], in_=wb16[:])
            d_all = singles.tile([128, C_TILES, u_pc], F32)

            # Absorb the weight-DMA sem waits into throwaway DVE ops so the
            # hot-loop instructions each need only one sync-wait.
            scr_a = singles.tile([128, 1], F32)
            nc.vector.tensor_copy(scr_a[:], w32_sb[:, 0, 0:1])
            scr_b = singles.tile([128, 1], BF16)
            nc.vector.tensor_copy(scr_b[:], wb16_sb[:, 0:1])

            # HAM warm-up: junk matmuls on the weights while S tiles land,
            # so the PE is at full clock when the real stream starts.
            junk = junkpool.tile([128, C_PAD], F32)
            for _ in range(6):
                nc.tensor.matmul(
                    junk[:],
                    lhsT=wt8_sb[:, 0:2, ts(0, 128)],
                    rhs=wt8_sb[:, 0:2, :],
                    start=True,
                    stop=True,
                    perf_mode=DR,
                )

            for j in range(u_pc):
                if j == 0:
                    s01_sb, s23_sb = s01_first, s23_first
                else:
                    s01_sb = s01pool.tile([128, 2, A], FP8, tag="s01")
                    nc.sync.dma_start(out=s01_sb[:], in_=s01[j])
                    s23_sb = s23pool.tile([128, 2, 256], FP8, tag="s23")
                    nc.sync.dma_start(out=s23_sb[:], in_=s23[j])
                for t in range(C_TILES):
                    p_ps = ppool.tile([128, A], F32, tag="ps")
                    # P[c, b] = sum_a Wt[a,c] M[a,b]; M's lower-left quarter
                    # is identically zero, so rows 256:512 only touch cols
                    # 256:512 (the b<256 / b>=256 col ranges are separate
                    # accumulation groups).
                    nc.tensor.matmul(
                        p_ps[:, 0:256],
                        lhsT=wt8_sb[:, 0:2, ts(t, 128)],
                        rhs=s01_sb[:, :, 0:256],
                        start=True,
                        stop=True,
                        perf_mode=DR,
                    )
                    nc.tensor.matmul(
                        p_ps[:, 256:512],
                        lhsT=wt8_sb[:, 0:2, ts(t, 128)],
                        rhs=s01_sb[:, :, 256:512],
                        start=True,
                        stop=False,
                        perf_mode=DR,
                    )
                    nc.tensor.matmul(
                        p_ps[:, 256:512],
                        lhsT=wt8_sb[:, 2:4, ts(t, 128)],
                        rhs=s23_sb[:],
                        start=False,
                        stop=True,
                        perf_mode=DR,
                    )
                    if t < 2:
                        # d[c] = sum_b P[c,b] * W[c,b], fused on the DVE
                        # straight out of PSUM (TensorScalarPtr; the
                        # TensorTensorReduce opcode kills the device).
                        o_sb = scrpool.tile([128, A], BF16, tag="o")
                        nc.vector.scalar_tensor_tensor(
                            out=o_sb[:],
                            in0=p_ps[:],
                            scalar=1.0,
                            in1=w32_sb[:, t, :],
                            op0=MULT,
                            op1=MULT,
                            accum_out=d_all[:, t, j : j + 1],
                        )
                    else:
                        # Third c-tile: ACT drains PSUM to SBUF (bf16), the
                        # otherwise-idle GPSIMD does the multiply, ACT
                        # accumulates the product over the free dim.
                        pc_sb = pcpool.tile([128, A], BF16, tag="pc")
                        nc.scalar.copy(out=pc_sb[:], in_=p_ps[:])
                        prod_sb = pcpool.tile([128, A], BF16, tag="prod")
                        nc.gpsimd.tensor_tensor(
                            out=prod_sb[:],
                            in0=pc_sb[:],
                            in1=wb16_sb[:],
                            op=MULT,
                        )
                        o2_sb = scrpool.tile([128, A], BF16, tag="o")
                        nc.scalar.activation(
                            out=o2_sb[:],
                            in_=prod_sb[:],
                            func=mybir.ActivationFunctionType.Copy,
                            accum_out=d_all[:, 2, j : j + 1],
                        )
            nc.sync.dma_start(out=dout[:], in_=d_all[:])
    nc.compile()
    return nc


def host_pack(fc_weight: np.ndarray, lab_pad: np.ndarray, cov: np.ndarray):
    """Build device inputs. Returns (wt8, w32, wb16, s01, s23, S_sym_f32)."""
    w_pad = np.zeros((C_PAD, A), np.float32)
    w_pad[:C] = fc_weight
    wt = np.ascontiguousarray(w_pad.T.reshape(4, 128, C_PAD).transpose(1, 0, 2))
    wt8 = (wt * W_SCALE).astype(FP8_NP)
    w_nat = w_pad.reshape(C_TILES, 128, A).transpose(1, 0, 2)  # [p, t, a]
    w32 = np.ascontiguousarray(w_nat[:, 0:2, :])
    wb16 = np.ascontiguousarray(w_nat[:, 2, :]).astype(ml_dtypes.bfloat16)

    sgath = cov[lab_pad]                       # [U_pad, A, A]
    s_sym = sgath + sgath.transpose(0, 2, 1)   # Sigma + Sigma^T, float32
    m = np.triu(s_sym) + np.triu(s_sym, 1)     # upper-tri fold, same quad form
    s01 = (
        m[:, 0:256, :].reshape(-1, 2, 128, A).transpose(0, 2, 1, 3) * S_SCALE
    ).astype(FP8_NP)
    s23 = (
        m[:, 256:512, 256:512].reshape(-1, 2, 128, 256).transpose(0, 2, 1, 3)
        * S_SCALE
    ).astype(FP8_NP)
    return wt8, w32, wb16, np.ascontiguousarray(s01), np.ascontiguousarray(s23), s_sym


_NC_CACHE: dict[int, bass.Bass] = {}


def _device_dS(fc_weight, uniq, cov):
    """Run the Bass kernel on 8 cores; returns (d_S [U, C] float64, S_sym [U,A,A])."""
    U = len(uniq)
    u_pc = math.ceil(U / N_CORES)
    u_pad = u_pc * N_CORES
    lab_pad = np.concatenate([uniq, np.full(u_pad - U, uniq[0], dtype=uniq.dtype)])
    wt8, w32, wb16, s01, s23, s_sym = host_pack(fc_weight, lab_pad, cov)

    if u_pc not in _NC_CACHE:
        _NC_CACHE[u_pc] = build_nc(u_pc)
    nc = _NC_CACHE[u_pc]

    in_maps = [
        {
            "wt8": wt8,
            "w32": w32,
            "wb16": wb16,
            "s01": np.ascontiguousarray(s01[i * u_pc : (i + 1) * u_pc]),
            "s23": np.ascontiguousarray(s23[i * u_pc : (i + 1) * u_pc]),
        }
        for i in range(N_CORES)
    ]
    res = run_bass_kernel_spmd(nc, in_maps, core_ids=list(range(N_CORES)))
    # dout[p, t, j] -> d[j, 128*t + p]
    d_s = np.concatenate(
        [r["dout"].transpose(2, 1, 0).reshape(u_pc, C_PAD) for r in res.results],
        axis=0,
    )[:U, :C]
    return d_s.astype(np.float64) / OUT_SCALE, s_sym[:U]


def kernel(
    fc_weight,
    features_source,
    y_s,
    labels_source,
    Lambda,
    mean_source,
    mean_target,
    covariance_target,
):
    fc_weight = np.asarray(fc_weight, dtype=np.float32)
    y_s = np.asarray(y_s, dtype=np.float32)
    labels = np.asarray(labels_source).astype(np.int64)
    lam = float(np.asarray(Lambda))
    mean_source = np.asarray(mean_source, dtype=np.float32)
    mean_target = np.asarray(mean_target, dtype=np.float32)
    cov = np.asarray(covariance_target, dtype=np.float32)

    n = labels.shape[0]
    uniq, inv = np.unique(labels, return_inverse=True)

    d_s, s_sym = _device_dS(fc_weight, uniq, cov)

    # Cheap per-unique-label terms in float64 on host.
    w64 = fc_weight.astype(np.float64)
    wl = w64[uniq]                                         # [U, A]
    wv = np.einsum("uab,ub->ua", s_sym.astype(np.float64), wl)  # S_l @ W_l
    b = wv @ w64.T                                         # [U, C]
    s = np.einsum("ua,ua->u", wl, wv)                      # W_l S_l W_l^T
    quad = 0.5 * d_s - b + 0.5 * s[:, None]                # [U, C]

    d_mean = (mean_target - mean_source).astype(np.float64)[uniq]  # [U, A]
    g = d_mean @ w64.T                                     # [U, C]
    g_self = np.einsum("ua,ua->u", wl, d_mean)             # [U]

    aug = (
        y_s.astype(np.float64)
        + 0.5 * lam * quad[inv]
        + lam * (g[inv] - g_self[inv][:, None])
    )
    mx = aug.max(axis=1, keepdims=True)
    lse = mx[:, 0] + np.log(np.exp(aug - mx).sum(axis=1))
    nll = lse - aug[np.arange(n), labels]
    return np.array(nll.mean(), dtype=np.float32)


# revision 6
# speedup vs baseline: 1.1006x; 1.1006x over previous
"""Trainium2 Bass kernel for nn_Cls_Loss_42331197670001.

Reference computation (N=128 samples, C=345 classes, A=512 features):
    dataW[n,c,:] = W[c] - W[labels[n]]
    sigma2[n,c]  = Lambda * dataW[n,c] @ Sigma[labels[n]] @ dataW[n,c]^T
    dW_dMean[n,c]= dataW[n,c] . (mean_target-mean_source)[labels[n]]
    aug = y_s + 0.5*sigma2 + Lambda*dW_dMean ;  loss = mean softmax-CE(aug, labels)

Everything depends on the sample n only through its label l, so the heavy
quadratic form is computed once per *unique* label:
    (W_c - W_l) Sigma_l (W_c - W_l)^T = 0.5*d_M(l,c) - b(l,c) + 0.5*s(l)
    d_M(l,c) = W_c M_l W_c^T          <- the only O(C*A*A) term, on device
    b, s, mean-shift, softmax-CE      <- tiny, host numpy in float64
where M_l = triu(S_l) + triu(S_l, 1) is the upper-triangular folding of the
symmetrized S_l = Sigma_l + Sigma_l^T (same quadratic form, 3/4 of the
matmul work at 256-row granularity since M[256:512, 0:256] == 0).

Device kernel (SPMD over 8 cores, unique labels sharded across cores),
per label j and per c-tile t (c = 345 classes in 3 tiles of 128):
    P[c,b] = sum_a Wt[a,c] M[a,b]    3 fp8 DoubleRow matmuls into PSUM
                                      (W^T stationary, shared across labels)
    d[c]   = sum_b P[c,b] W[c,b]     free-dim fused multiply-reduce:
      t=0,1: DVE scalar_tensor_tensor straight out of PSUM (fp32 in1;
             TensorTensorReduce and bf16 in1 both misbehave on real HW)
      t=2:   ACT copy PSUM->SBUF bf16 -> GPSIMD multiply -> DVE reduce
One DMA per label (the M pieces are packed into one [128,2,768] tile) keeps
the Sync sequencer off the critical path; memset-fed junk matmuls at t=0
warm the PE HAM clock gate before the first real matmul lands.
"""

import math
import sys

import numpy as np

try:
    import concourse.bass as bass
except ImportError:  # harness runs from a bare directory
    sys.path.insert(0, "/opt/trn_rl_repo")
    import concourse.bass as bass

import ml_dtypes

import concourse.mybir as mybir
import concourse.tile as tile
from concourse import bacc
from concourse.bass import ts
from concourse.bass_utils import run_bass_kernel_spmd

N_CORES = 8
A = 512          # feature dim
C = 345          # class count
C_PAD = 384      # 3 * 128
C_TILES = 3

W_SCALE = 16.0
S_SCALE = 32.0
OUT_SCALE = W_SCALE * S_SCALE

FP8 = mybir.dt.float8e4
BF16 = mybir.dt.bfloat16
F32 = mybir.dt.float32
FP8_NP = ml_dtypes.float8_e4m3

MULT = mybir.AluOpType.mult
ADD = mybir.AluOpType.add
DR = mybir.MatmulPerfMode.DoubleRow
AX_X = mybir.AxisListType.X


def build_nc(u_pc: int) -> bass.Bass:
    """Per core: u_pc labels; dout[p, t, j] = (W M_j W^T)[c,c] * OUT_SCALE
    for class c = 128*t + p."""
    nc = bacc.Bacc()
    wt8 = nc.dram_tensor("wt8", [128, 4, C_PAD], FP8, kind="ExternalInput")
    w32 = nc.dram_tensor("w32", [128, 2, A], F32, kind="ExternalInput")
    wb16 = nc.dram_tensor("wb16", [128, A], BF16, kind="ExternalInput")
    # per label: [:, :, 0:512] = M rows 0:256 (all cols);
    #            [:, :, 512:768] = M rows 256:512, cols 256:512
    sg = nc.dram_tensor("sg", [u_pc, 128, 2, 768], FP8, kind="ExternalInput")
    dout = nc.dram_tensor("dout", [128, C_TILES, u_pc], F32, kind="ExternalOutput")

    with tile.TileContext(nc) as tc:
        with (
            tc.tile_pool(name="singles", bufs=1) as singles,
            tc.tile_pool(name="spool", bufs=5) as spool,
            tc.tile_pool(name="scr", bufs=4) as scrpool,
            tc.tile_pool(name="pcopy", bufs=3) as pcpool,
            tc.tile_pool(name="psum", bufs=6, space="PSUM") as ppool,
            tc.tile_pool(name="junkp", bufs=1, space="PSUM") as junkpool,
        ):
            wt8_sb = singles.tile([128, 4, C_PAD], FP8)
            nc.sync.dma_start(out=wt8_sb[:], in_=wt8[:])
            # Junk warmup matmuls on the weights while S tiles land, so the
            # PE HAM clock-gate opens (K=8/8) before the real stream starts.
            junk = junkpool.tile([128, C_PAD], F32)
            for _ in range(8):
                nc.tensor.matmul(
                    junk[:], lhsT=wt8_sb[:, 0:2, ts(0, 128)],
                    rhs=wt8_sb[:, 0:2, :], start=True, stop=True,
                    perf_mode=DR,
                )
            s_first = spool.tile([128, 2, 768], FP8, tag="s")
            nc.sync.dma_start(out=s_first[:], in_=sg[0])
            s_second = None
            if u_pc > 1:
                s_second = spool.tile([128, 2, 768], FP8, tag="s")
                nc.sync.dma_start(out=s_second[:], in_=sg[1])
            w32_sb = singles.tile([128, 2, A], F32)
            nc.sync.dma_start(out=w32_sb[:], in_=w32[:])
            wb16_sb = singles.tile([128, A], BF16)
            nc.sync.dma_start(out=wb16_sb[:], in_=wb16[:])
            d_all = singles.tile([128, C_TILES, u_pc], F32)

            # Absorb weight-DMA sem waits into throwaway DVE ops so hot-loop
            # instructions each need only one sync-wait.
            scr_a = singles.tile([128, 1], F32)
            nc.vector.tensor_copy(scr_a[:], w32_sb[:, 0, 0:1])
            scr_b = singles.tile([128, 1], BF16)
            nc.vector.tensor_copy(scr_b[:], wb16_sb[:, 0:1])
            scr_c = singles.tile([128, 1], BF16)
            nc.vector.tensor_copy(scr_c[:], wt8_sb[:, 0, 0:1])

            pending = None  # (prod_sb, j) for the previous label's t2 reduce
            for j in range(u_pc):
                if j == 0:
                    s_sb = s_first
                elif j == 1:
                    s_sb = s_second
                else:
                    s_sb = spool.tile([128, 2, 768], FP8, tag="s")
                    nc.sync.dma_start(out=s_sb[:], in_=sg[j])
                for t in range(C_TILES):
                    p_ps = ppool.tile([128, A], F32, tag="ps")
                    # P[c, b] = sum_a Wt[a,c] M[a,b]; M's lower-left quarter
                    # is identically zero, so rows 256:512 only touch cols
                    # 256:512 (separate accumulation groups per col range).
                    nc.tensor.matmul(
                        p_ps[:, 0:256],
                        lhsT=wt8_sb[:, 0:2, ts(t, 128)],
                        rhs=s_sb[:, :, 0:256],
                        start=True, stop=True, perf_mode=DR,
                    )
                    nc.tensor.matmul(
                        p_ps[:, 256:512],
                        lhsT=wt8_sb[:, 0:2, ts(t, 128)],
                        rhs=s_sb[:, :, 256:512],
                        start=True, stop=False, perf_mode=DR,
                    )
                    nc.tensor.matmul(
                        p_ps[:, 256:512],
                        lhsT=wt8_sb[:, 2:4, ts(t, 128)],
                        rhs=s_sb[:, :, 512:768],
                        start=False, stop=True, perf_mode=DR,
                    )
                    if t < 2:
                        # d[c] = sum_b P[c,b] * W[c,b], fused on the DVE
                        # straight out of PSUM.
                        o_sb = scrpool.tile([128, A], BF16, tag="o")
                        nc.vector.scalar_tensor_tensor(
                            out=o_sb[:],
                            in0=p_ps[:],
                            scalar=1.0,
                            in1=w32_sb[:, t, :],
                            op0=MULT,
                            op1=MULT,
                            accum_out=d_all[:, t, j : j + 1],
                        )
                    else:
                        # Third c-tile: ACT drains PSUM to SBUF (bf16), the
                        # otherwise-idle GPSIMD multiplies, DVE reduces.
                        pc_sb = pcpool.tile([128, A], BF16, tag="pc")
                        nc.scalar.copy(out=pc_sb[:], in_=p_ps[:])
                        prod_sb = pcpool.tile([128, A], BF16, tag="prod")
                        nc.gpsimd.tensor_tensor(
                            out=prod_sb[:], in0=pc_sb[:], in1=wb16_sb[:],
                            op=MULT,
                        )
                        if pending is not None:
                            po, pj = pending
                            nc.vector.tensor_reduce(
                                out=d_all[:, 2, pj : pj + 1], in_=po[:],
                                axis=AX_X, op=ADD,
                            )
                        pending = (prod_sb, j)
            po, pj = pending
            nc.vector.tensor_reduce(
                out=d_all[:, 2, pj : pj + 1], in_=po[:], axis=AX_X, op=ADD,
            )
            nc.sync.dma_start(out=dout[:], in_=d_all[:])
    nc.compile()
    return nc


def host_pack(fc_weight: np.ndarray, lab_pad: np.ndarray, cov: np.ndarray):
    """Build device inputs. Returns (wt8, w32, wb16, sg, S_sym_f32)."""
    w_pad = np.zeros((C_PAD, A), np.float32)
    w_pad[:C] = fc_weight
    wt = np.ascontiguousarray(w_pad.T.reshape(4, 128, C_PAD).transpose(1, 0, 2))
    wt8 = (wt * W_SCALE).astype(FP8_NP)
    w_nat = w_pad.reshape(C_TILES, 128, A).transpose(1, 0, 2)  # [p, t, a]
    w32 = np.ascontiguousarray(w_nat[:, 0:2, :])
    wb16 = np.ascontiguousarray(w_nat[:, 2, :]).astype(ml_dtypes.bfloat16)

    sgath = cov[lab_pad]                       # [U_pad, A, A]
    s_sym = sgath + sgath.transpose(0, 2, 1)   # Sigma + Sigma^T, float32
    m = np.triu(s_sym) + np.triu(s_sym, 1)     # upper-tri fold, same quad form
    n_lab = len(lab_pad)
    sg = np.empty((n_lab, 128, 2, 768), FP8_NP)
    sg[:, :, :, 0:512] = (
        m[:, 0:256, :].reshape(-1, 2, 128, A).transpose(0, 2, 1, 3) * S_SCALE
    ).astype(FP8_NP)
    sg[:, :, :, 512:768] = (
        m[:, 256:512, 256:512].reshape(-1, 2, 128, 256).transpose(0, 2, 1, 3)
        * S_SCALE
    ).astype(FP8_NP)
    return wt8, w32, wb16, sg, s_sym


_NC_CACHE: dict[int, bass.Bass] = {}


def _device_dS(fc_weight, uniq, cov):
    """Run the Bass kernel on 8 cores; returns (d_S [U, C] float64, S_sym [U,A,A])."""
    U = len(uniq)
    u_pc = math.ceil(U / N_CORES)
    u_pad = u_pc * N_CORES
    lab_pad = np.concatenate([uniq, np.full(u_pad - U, uniq[0], dtype=uniq.dtype)])
    wt8, w32, wb16, sg, s_sym = host_pack(fc_weight, lab_pad, cov)

    if u_pc not in _NC_CACHE:
        _NC_CACHE[u_pc] = build_nc(u_pc)
    nc = _NC_CACHE[u_pc]

    in_maps = [
        {
            "wt8": wt8,
            "w32": w32,
            "wb16": wb16,
            "sg": np.ascontiguousarray(sg[i * u_pc : (i + 1) * u_pc]),
        }
        for i in range(N_CORES)
    ]
    res = run_bass_kernel_spmd(nc, in_maps, core_ids=list(range(N_CORES)))
    # dout[p, t, j] -> d[j, 128*t + p]
    d_s = np.concatenate(
        [r["dout"].transpose(2, 1, 0).reshape(u_pc, C_PAD) for r in res.results],
        axis=0,
    )[:U, :C]
    return d_s.astype(np.float64) / OUT_SCALE, s_sym[:U]


def kernel(
    fc_weight,
    features_source,
    y_s,
    labels_source,
    Lambda,
    mean_source,
    mean_target,
    covariance_target,
):
    fc_weight = np.asarray(fc_weight, dtype=np.float32)
    y_s = np.asarray(y_s, dtype=np.float32)
    labels = np.asarray(labels_source).astype(np.int64)
    lam = float(np.asarray(Lambda))
    mean_source = np.asarray(mean_source, dtype=np.float32)
    mean_target = np.asarray(mean_target, dtype=np.float32)
    cov = np.asarray(covariance_target, dtype=np.float32)

    n = labels.shape[0]
    uniq, inv = np.unique(labels, return_inverse=True)

    d_s, s_sym = _device_dS(fc_weight, uniq, cov)

    # Cheap per-unique-label terms in float64 on host.
    w64 = fc_weight.astype(np.float64)
    wl = w64[uniq]                                         # [U, A]
    wv = np.einsum("uab,ub->ua", s_sym.astype(np.float64), wl)  # S_l @ W_l
    b = wv @ w64.T                                         # [U, C]
    s = np.einsum("ua,ua->u", wl, wv)                      # W_l S_l W_l^T
    quad = 0.5 * d_s - b + 0.5 * s[:, None]                # [U, C]

    d_mean = (mean_target - mean_source).astype(np.float64)[uniq]  # [U, A]
    g = d_mean @ w64.T                                     # [U, C]
    g_self = np.einsum("ua,ua->u", wl, d_mean)             # [U]

    aug = (
        y_s.astype(np.float64)
        + 0.5 * lam * quad[inv]
        + lam * (g[inv] - g_self[inv][:, None])
    )
    mx = aug.max(axis=1, keepdims=True)
    lse = mx[:, 0] + np.log(np.exp(aug - mx).sum(axis=1))
    nll = lse - aug[np.arange(n), labels]
    return np.array(nll.mean(), dtype=np.float32)


# revision 7
# speedup vs baseline: 1.2246x; 1.1127x over previous
"""Trainium2 Bass kernel for nn_Cls_Loss_42331197670001.

Reference computation (N=128 samples, C=345 classes, A=512 features):
    dataW[n,c,:] = W[c] - W[labels[n]]
    sigma2[n,c]  = Lambda * dataW[n,c] @ Sigma[labels[n]] @ dataW[n,c]^T
    dW_dMean[n,c]= dataW[n,c] . (mean_target-mean_source)[labels[n]]
    aug = y_s + 0.5*sigma2 + Lambda*dW_dMean ;  loss = mean softmax-CE(aug, labels)

Everything depends on the sample n only through its label l, so the heavy
quadratic form is computed once per *unique* label:
    (W_c - W_l) Sigma_l (W_c - W_l)^T = 0.5*d_M(l,c) - b(l,c) + 0.5*s(l)
    d_M(l,c) = W_c M_l W_c^T          <- the only O(C*A*A) term, on device
    b, s, mean-shift, softmax-CE      <- tiny, host numpy in float64
where M_l = triu(S_l) + triu(S_l, 1) is the upper-triangular folding of the
symmetrized S_l = Sigma_l + Sigma_l^T (same quadratic form, 3/4 of the
matmul work at 256-row granularity since M[256:512, 0:256] == 0).

Device kernel (SPMD over 8 cores, unique labels sharded across cores),
per label j and per c-tile t (c = 345 classes in 3 tiles of 128):
    P[c,b] = sum_a Wt[a,c] M[a,b]    3 fp8 DoubleRow matmuls into PSUM
                                      (W^T stationary, shared across labels)
    d[c]   = sum_b P[c,b] W[c,b]     free-dim fused multiply-reduce:
      t=0,1: DVE scalar_tensor_tensor straight out of PSUM (fp32 in1;
             TensorTensorReduce and bf16 in1 both misbehave on real HW)
      t=2:   ACT copy PSUM->SBUF bf16 -> GPSIMD multiply -> DVE reduce
One DMA per label (the M pieces are packed into one [128,2,768] tile) keeps
the Sync sequencer off the critical path; memset-fed junk matmuls at t=0
warm the PE HAM clock gate before the first real matmul lands.
"""

import math
import sys

import numpy as np

try:
    import concourse.bass as bass
except ImportError:  # harness runs from a bare directory
    sys.path.insert(0, "/opt/trn_rl_repo")
    import concourse.bass as bass

import ml_dtypes

import concourse.mybir as mybir
import concourse.tile as tile
from concourse import bacc
from concourse.bass import ts
from concourse.bass_utils import run_bass_kernel_spmd

N_CORES = 8
A = 512          # feature dim
C = 345          # class count
C_PAD = 384      # 3 * 128
C_TILES = 3

W_SCALE = 16.0
S_SCALE = 32.0
OUT_SCALE = W_SCALE * S_SCALE

FP8 = mybir.dt.float8e4
BF16 = mybir.dt.bfloat16
F32 = mybir.dt.float32
FP8_NP = ml_dtypes.float8_e4m3

MULT = mybir.AluOpType.mult
ADD = mybir.AluOpType.add
DR = mybir.MatmulPerfMode.DoubleRow
AX_X = mybir.AxisListType.X


def build_nc(u_pc: int) -> bass.Bass:
    """Per core: u_pc labels; dout[p, t, j] = (W M_j W^T)[c,c] * OUT_SCALE
    for class c = 128*t + p."""
    nc = bacc.Bacc()
    wt8 = nc.dram_tensor("wt8", [128, 4, C_PAD], FP8, kind="ExternalInput")
    w32 = nc.dram_tensor("w32", [128, 2, A], F32, kind="ExternalInput")
    wb16 = nc.dram_tensor("wb16", [128, A], BF16, kind="ExternalInput")
    # per label: [:, :, 0:512] = M rows 0:256 (all cols);
    #            [:, :, 512:768] = M rows 256:512, cols 256:512
    sg = nc.dram_tensor("sg", [u_pc, 128, 2, 768], FP8, kind="ExternalInput")
    dout = nc.dram_tensor("dout", [128, C_TILES, u_pc], F32, kind="ExternalOutput")

    with tile.TileContext(nc) as tc:
        with (
            tc.tile_pool(name="singles", bufs=1) as singles,
            tc.tile_pool(name="spool", bufs=5) as spool,
            tc.tile_pool(name="scr", bufs=4) as scrpool,
            tc.tile_pool(name="pcopy", bufs=3) as pcpool,
            tc.tile_pool(name="psum", bufs=6, space="PSUM") as ppool,
            tc.tile_pool(name="junkp", bufs=1, space="PSUM") as junkpool,
        ):
            wt8_sb = singles.tile([128, 4, C_PAD], FP8)
            nc.sync.dma_start(out=wt8_sb[:], in_=wt8[:])
            # Junk warmup matmuls on the weights while S tiles land, so the
            # PE HAM clock-gate opens (K=8/8) before the real stream starts.
            junk = junkpool.tile([128, C_PAD], F32)
            for _ in range(8):
                nc.tensor.matmul(
                    junk[:], lhsT=wt8_sb[:, 0:2, ts(0, 128)],
                    rhs=wt8_sb[:, 0:2, :], start=True, stop=True,
                    perf_mode=DR,
                )
            s_first = spool.tile([128, 2, 768], FP8, tag="s")
            nc.sync.dma_start(out=s_first[:], in_=sg[0])
            s_second = None
            if u_pc > 1:
                s_second = spool.tile([128, 2, 768], FP8, tag="s")
                nc.sync.dma_start(out=s_second[:], in_=sg[1])
            w32_sb = singles.tile([128, 2, A], F32)
            nc.sync.dma_start(out=w32_sb[:], in_=w32[:])
            wb16_sb = singles.tile([128, A], BF16)
            nc.sync.dma_start(out=wb16_sb[:], in_=wb16[:])
            d_all = singles.tile([128, C_TILES, u_pc], F32)

            # Absorb weight-DMA sem waits into throwaway DVE ops so hot-loop
            # instructions each need only one sync-wait.
            scr_a = singles.tile([128, 1], F32)
            nc.vector.tensor_copy(scr_a[:], w32_sb[:, 0, 0:1])
            scr_b = singles.tile([128, 1], BF16)
            nc.vector.tensor_copy(scr_b[:], wb16_sb[:, 0:1])
            scr_c = singles.tile([128, 1], BF16)
            nc.vector.tensor_copy(scr_c[:], wt8_sb[:, 0, 0:1])

            pending = None  # (prod_sb, j) for the previous label's t2 reduce
            for j in range(u_pc):
                if j == 0:
                    s_sb = s_first
                elif j == 1:
                    s_sb = s_second
                else:
                    s_sb = spool.tile([128, 2, 768], FP8, tag="s")
                    nc.sync.dma_start(out=s_sb[:], in_=sg[j])
                for t in range(C_TILES):
                    p_ps = ppool.tile([128, A], F32, tag="ps")
                    # P[c, b] = sum_a Wt[a,c] M[a,b]; M's lower-left quarter
                    # is identically zero, so rows 256:512 only touch cols
                    # 256:512 (separate accumulation groups per col range).
                    nc.tensor.matmul(
                        p_ps[:, 0:256],
                        lhsT=wt8_sb[:, 0:2, ts(t, 128)],
                        rhs=s_sb[:, :, 0:256],
                        start=True, stop=True, perf_mode=DR,
                    )
                    nc.tensor.matmul(
                        p_ps[:, 256:512],
                        lhsT=wt8_sb[:, 0:2, ts(t, 128)],
                        rhs=s_sb[:, :, 256:512],
                        start=True, stop=False, perf_mode=DR,
                    )
                    nc.tensor.matmul(
                        p_ps[:, 256:512],
                        lhsT=wt8_sb[:, 2:4, ts(t, 128)],
                        rhs=s_sb[:, :, 512:768],
                        start=False, stop=True, perf_mode=DR,
                    )
                    if t < 2:
                        # d[c] = sum_b P[c,b] * W[c,b], fused on the DVE
                        # straight out of PSUM.
                        o_sb = scrpool.tile([128, A], BF16, tag="o")
                        nc.vector.scalar_tensor_tensor(
                            out=o_sb[:],
                            in0=p_ps[:],
                            scalar=1.0,
                            in1=w32_sb[:, t, :],
                            op0=MULT,
                            op1=MULT,
                            accum_out=d_all[:, t, j : j + 1],
                        )
                    else:
                        # Third c-tile: ACT drains PSUM to SBUF (bf16), the
                        # otherwise-idle GPSIMD multiplies, DVE reduces.
                        pc_sb = pcpool.tile([128, A], BF16, tag="pc")
                        nc.scalar.copy(out=pc_sb[:], in_=p_ps[:])
                        if pending is not None:
                            # Previous label's product sum, on the ACT
                            # accumulator — emitted after this label's copy
                            # so the ACT FIFO never stalls on the GP mult.
                            po, pj = pending
                            o2_sb = scrpool.tile([128, A], BF16, tag="o2")
                            nc.scalar.activation(
                                out=o2_sb[:], in_=po[:],
                                func=mybir.ActivationFunctionType.Copy,
                                accum_out=d_all[:, 2, pj : pj + 1],
                            )
                        prod_sb = pcpool.tile([128, A], BF16, tag="prod")
                        nc.gpsimd.tensor_tensor(
                            out=prod_sb[:], in0=pc_sb[:], in1=wb16_sb[:],
                            op=MULT,
                        )
                        pending = (prod_sb, j)
            po, pj = pending
            o2_sb = scrpool.tile([128, A], BF16, tag="o2")
            nc.scalar.activation(
                out=o2_sb[:], in_=po[:],
                func=mybir.ActivationFunctionType.Copy,
                accum_out=d_all[:, 2, pj : pj + 1],
            )
            nc.sync.dma_start(out=dout[:], in_=d_all[:])
    nc.compile()
    return nc


def host_pack(fc_weight: np.ndarray, lab_pad: np.ndarray, cov: np.ndarray):
    """Build device inputs. Returns (wt8, w32, wb16, sg, S_sym_f32)."""
    w_pad = np.zeros((C_PAD, A), np.float32)
    w_pad[:C] = fc_weight
    wt = np.ascontiguousarray(w_pad.T.reshape(4, 128, C_PAD).transpose(1, 0, 2))
    wt8 = (wt * W_SCALE).astype(FP8_NP)
    w_nat = w_pad.reshape(C_TILES, 128, A).transpose(1, 0, 2)  # [p, t, a]
    w32 = np.ascontiguousarray(w_nat[:, 0:2, :])
    wb16 = np.ascontiguousarray(w_nat[:, 2, :]).astype(ml_dtypes.bfloat16)

    sgath = cov[lab_pad]                       # [U_pad, A, A]
    s_sym = sgath + sgath.transpose(0, 2, 1)   # Sigma + Sigma^T, float32
    m = np.triu(s_sym) + np.triu(s_sym, 1)     # upper-tri fold, same quad form
    n_lab = len(lab_pad)
    sg = np.empty((n_lab, 128, 2, 768), FP8_NP)
    sg[:, :, :, 0:512] = (
        m[:, 0:256, :].reshape(-1, 2, 128, A).transpose(0, 2, 1, 3) * S_SCALE
    ).astype(FP8_NP)
    sg[:, :, :, 512:768] = (
        m[:, 256:512, 256:512].reshape(-1, 2, 128, 256).transpose(0, 2, 1, 3)
        * S_SCALE
    ).astype(FP8_NP)
    return wt8, w32, wb16, sg, s_sym


_NC_CACHE: dict[int, bass.Bass] = {}


def _device_dS(fc_weight, uniq, cov):
    """Run the Bass kernel on 8 cores; returns (d_S [U, C] float64, S_sym [U,A,A])."""
    U = len(uniq)
    u_pc = math.ceil(U / N_CORES)
    u_pad = u_pc * N_CORES
    lab_pad = np.concatenate([uniq, np.full(u_pad - U, uniq[0], dtype=uniq.dtype)])
    wt8, w32, wb16, sg, s_sym = host_pack(fc_weight, lab_pad, cov)

    if u_pc not in _NC_CACHE:
        _NC_CACHE[u_pc] = build_nc(u_pc)
    nc = _NC_CACHE[u_pc]

    in_maps = [
        {
            "wt8": wt8,
            "w32": w32,
            "wb16": wb16,
            "sg": np.ascontiguousarray(sg[i * u_pc : (i + 1) * u_pc]),
        }
        for i in range(N_CORES)
    ]
    res = run_bass_kernel_spmd(nc, in_maps, core_ids=list(range(N_CORES)))
    # dout[p, t, j] -> d[j, 128*t + p]
    d_s = np.concatenate(
        [r["dout"].transpose(2, 1, 0).reshape(u_pc, C_PAD) for r in res.results],
        axis=0,
    )[:U, :C]
    return d_s.astype(np.float64) / OUT_SCALE, s_sym[:U]


def kernel(
    fc_weight,
    features_source,
    y_s,
    labels_source,
    Lambda,
    mean_source,
    mean_target,
    covariance_target,
):
    fc_weight = np.asarray(fc_weight, dtype=np.float32)
    y_s = np.asarray(y_s, dtype=np.float32)
    labels = np.asarray(labels_source).astype(np.int64)
    lam = float(np.asarray(Lambda))
    mean_source = np.asarray(mean_source, dtype=np.float32)
    mean_target = np.asarray(mean_target, dtype=np.float32)
    cov = np.asarray(covariance_target, dtype=np.float32)

    n = labels.shape[0]
    uniq, inv = np.unique(labels, return_inverse=True)

    d_s, s_sym = _device_dS(fc_weight, uniq, cov)

    # Cheap per-unique-label terms in float64 on host.
    w64 = fc_weight.astype(np.float64)
    wl = w64[uniq]                                         # [U, A]
    wv = np.einsum("uab,ub->ua", s_sym.astype(np.float64), wl)  # S_l @ W_l
    b = wv @ w64.T                                         # [U, C]
    s = np.einsum("ua,ua->u", wl, wv)                      # W_l S_l W_l^T
    quad = 0.5 * d_s - b + 0.5 * s[:, None]                # [U, C]

    d_mean = (mean_target - mean_source).astype(np.float64)[uniq]  # [U, A]
    g = d_mean @ w64.T                                     # [U, C]
    g_self = np.einsum("ua,ua->u", wl, d_mean)             # [U]

    aug = (
        y_s.astype(np.float64)
        + 0.5 * lam * quad[inv]
        + lam * (g[inv] - g_self[inv][:, None])
    )
    mx = aug.max(axis=1, keepdims=True)
    lse = mx[:, 0] + np.log(np.exp(aug - mx).sum(axis=1))
    nll = lse - aug[np.arange(n), labels]
    return np.array(nll.mean(), dtype=np.float32)
